# revision 2
# baseline (speedup 1.0000x reference)
"""Trainium2 Bass kernel for nn_BilinearInterpolator (dense per-coord CNN).

Math (per (b, n) pair):
  u      = w1[:, :5] @ [image_b; pos]              # [64, 1024], shared over n
  v      = w1[:, 5:] @ coords[b, n] + b1           # [64] per-pair bias
  h1     = leaky(u + v)                            # [64, 1024]
  h_l    = leaky(W_l h_{l-1} + b_l)   l = 2..5
  pooled = mean_hw(h5);  out = sigmoid(wl @ pooled + bl)

Sharding: 512 (b, n) pairs data-parallel over 8 cores (64 pairs each; every
core owns a single b). On-chip layout packs 2 pairs per 128-partition tile
(channels 0-63 = even pair, 64-127 = odd pair); all matmuls use block-diagonal
[128, 128] fp16 weights.

Engine split (v2): the elementwise PSUM->SBUF crossings are the wall-clock
bottleneck, so they are divided to keep ScalarE and VectorE both ~100% busy:
  - L2..L4: ScalarE Prelu (bias+leaky fused, one pass) over pack-PAIR units
    [128, 2048] spanning 4 PSUM banks, amortizing the per-op fixed cost.
    A few tunable units (D24) run on VectorE instead.
  - L1 (SBUF-only): VectorE, fp16 4x-mode add + 2x-mode stt leaky.
  - L5: VectorE, one 1x bias-add from PSUM per unit + two 2x stt leaky ops
    whose accum_out yields the spatial sum (pooling) for free.
PSUM holds two [128, 2048] f32 unit slots (all 8 banks) that the matmul
stream and the crossings rotate through; emission is a skewed wavefront so
all layers stay in flight and engine FIFOs interleave independent units.
"""

import sys

if "/opt/trn_rl_repo" not in sys.path:
    sys.path.insert(0, "/opt/trn_rl_repo")

import ml_dtypes
import numpy as np

import concourse.mybir as mybir
from concourse.bacc import Bacc
from concourse import tile
from concourse.bass_utils import run_bass_kernel_spmd

B, N, H, W, C = 4, 128, 32, 32, 64
HW = H * W
NCORES = 8
PAIRS = (B * N) // NCORES  # 64 pairs per core
PACKS = PAIRS // 2  # 32 packed tiles per core
UNITS = PACKS // 2  # 16 pack-pair units per core
NEG = 0.1
F32 = mybir.dt.float32
F16 = mybir.dt.float16
MM_DT = F16

A = mybir.ActivationFunctionType
OP = mybir.AluOpType

SK = 2  # wavefront skew (waves) between consecutive layers
# L2-4 pack-pair units owned by VectorE instead of ScalarE (load balance).
D24 = {(3, 5), (4, 11)}


def _build():
    nc = Bacc()
    d = {}
    for name, shape, dt in [
        ("xin", [5, HW], F16),
        ("crd", [4, PACKS], F16),
        ("wu", [5, 128], F16),
        ("wc", [4, 128], F16),
        ("bball", [128, 4], F32),
        ("bb1", [128, 1], F32),
        ("wh", [128, 6], F32),
        ("bbl", [6, 1], F32),
        ("wall", [128, 4 * 128], MM_DT),
    ]:
        d[name] = nc.dram_tensor(name, shape, dt, kind="ExternalInput")
    out_d = nc.dram_tensor("out", [6, PACKS], F32, kind="ExternalOutput")

    with tile.TileContext(nc) as tc:
        with (
            tc.tile_pool(name="consts", bufs=1) as consts,
            tc.tile_pool(name="h1pool", bufs=7) as h1pool,
            tc.tile_pool(name="apool", bufs=5) as apool,
            tc.tile_pool(name="hpool", bufs=10) as hpool,
            tc.tile_pool(name="ypool", bufs=3) as ypool,
            tc.tile_pool(name="spool", bufs=4) as spool,
            tc.tile_pool(name="zpool", bufs=2, space="PSUM") as zpool,
        ):
            sb = {}
            for name in d:
                sb[name] = consts.tile(list(d[name].shape), d[name].dtype, tag=name, name="sb_" + name)
                nc.sync.dma_start(sb[name][:], d[name][:])

            w_l = {l: sb["wall"][:, 128 * (l - 2) : 128 * (l - 1)] for l in (2, 3, 4, 5)}
            bb_l = {l: sb["bball"][:, (l - 2) : (l - 1)] for l in (2, 3, 4, 5)}

            # per-pair input bias (layer-1 ops need it earliest)
            zpc = zpool.tile([128, PACKS], F32, tag="z")
            nc.tensor.matmul(zpc[:], sb["wc"][:], sb["crd"][:])
            bias1 = consts.tile([128, PACKS], F32, tag="bias1")
            nc.scalar.activation(bias1[:], zpc[:], A.Identity, bias=sb["bb1"][:])

            # u = first conv applied to [image; pos], duplicated to both
            # partition halves by the doubled-column lhsT; fp16 so layer-1
            # adds run in the DVE 4x mode. Copy halves on both engines.
            zpu = zpool.tile([128, HW], F32, tag="z")
            nc.tensor.matmul(zpu[:, 0:512], sb["wu"][:], sb["xin"][:, 0:512])
            nc.tensor.matmul(zpu[:, 512:1024], sb["wu"][:], sb["xin"][:, 512:1024])
            u_dup = consts.tile([128, HW], F16, tag="u_dup")
            nc.scalar.copy(u_dup[:, 0:512], zpu[:, 0:512])
            nc.vector.tensor_scalar(
                u_dup[:, 512:1024], zpu[:, 512:1024], 1.0, None, OP.mult
            )

            pooled = consts.tile([128, PACKS], F32, tag="pooled")

            h1 = {}
            hcur = {}

            def emit_l1(t):
                a = apool.tile([128, HW], F16, tag="a", name=f"a1_{t}")
                nc.vector.tensor_scalar(
                    a[:], u_dup[:], bias1[:, t : t + 1], None, OP.add
                )
                h = h1pool.tile([128, HW], F16, tag="h1", name=f"h1_{t}")
                nc.vector.scalar_tensor_tensor(h[:], a[:], NEG, a[:], OP.mult, OP.max)
                h1[t] = h

            def emit_unit(l, p):
                z = zpool.tile([128, 2 * HW], F32, tag="z", name=f"z{l}_{p}")
                if l == 2:
                    srcs = [
                        (h1[2 * p], 0), (h1[2 * p], 512),
                        (h1[2 * p + 1], 0), (h1[2 * p + 1], 512),
                    ]
                else:
                    prev = hcur[(l - 1, p)]
                    srcs = [(prev, 0), (prev, 512), (prev, 1024), (prev, 1536)]
                for i, (src, c0) in enumerate(srcs):
                    nc.tensor.matmul(
                        z[:, 512 * i : 512 * (i + 1)], w_l[l], src[:, c0 : c0 + 512],
                        start=True, stop=True, skip_group_check=True,
                    )
                if l == 5:
                    y = ypool.tile([128, 2 * HW], F16, tag="y", name=f"y5_{p}")
                    nc.vector.tensor_scalar(y[:], z[:], bb_l[5], None, OP.add)
                    for k in (0, 1):
                        t = 2 * p + k
                        s = spool.tile([128, HW], F16, tag="s", name=f"s5_{t}")
                        nc.vector.scalar_tensor_tensor(
                            s[:], y[:, HW * k : HW * (k + 1)], NEG,
                            y[:, HW * k : HW * (k + 1)], OP.mult, OP.max,
                            accum_out=pooled[:, t : t + 1],
                        )
                elif (l, p) in D24:
                    y = ypool.tile([128, 2 * HW], F16, tag="y", name=f"y{l}_{p}")
                    nc.vector.tensor_scalar(y[:], z[:], bb_l[l], None, OP.add)
                    h = hpool.tile([128, 2 * HW], F16, tag="h", name=f"h{l}_{p}")
                    nc.vector.scalar_tensor_tensor(
                        h[:], y[:], NEG, y[:], OP.mult, OP.max
                    )
                    hcur[(l, p)] = h
                else:
                    h = hpool.tile([128, 2 * HW], F16, tag="h", name=f"h{l}_{p}")
                    nc.scalar.activation(
                        h[:], z[:], A.Prelu, bias=bb_l[l], scale=1.0, alpha=NEG
                    )
                    hcur[(l, p)] = h

            for w in range(UNITS + SK * 4):
                if w < UNITS:
                    emit_l1(2 * w)
                    emit_l1(2 * w + 1)
                for l in (2, 3, 4, 5):
                    p = w - SK * (l - 1)
                    if 0 <= p < UNITS:
                        emit_unit(l, p)

            # ---- head ----
            zph = zpool.tile([6, PACKS], F32, tag="z")
            nc.tensor.matmul(zph[:], sb["wh"][:], pooled[:])
            out_sb = consts.tile([6, PACKS], F32, tag="out_sb")
            nc.scalar.activation(out_sb[:], zph[:], A.Sigmoid, bias=sb["bbl"][:])
            nc.sync.dma_start(out_d[:], out_sb[:])

    nc.compile()
    return nc


_CACHE = {}


def _get_nc():
    if "nc" not in _CACHE:
        _CACHE["nc"] = _build()
    return _CACHE["nc"]


def _prep_core_inputs(image, coords, w1, b1, ws, bs, wl, bl, core):
    b = core // 2
    n0 = (core % 2) * PAIRS

    row = (np.arange(H, dtype=np.float32) / (H - 1))[:, None] * np.ones(
        (1, W), np.float32
    )
    col = np.ones((H, 1), np.float32) * (np.arange(W, dtype=np.float32) / (W - 1))[None]
    pos = np.stack([row, col], 0).reshape(2, HW)
    xin = np.concatenate([image[b].reshape(3, HW), pos], 0)

    cs = coords[b, n0 : n0 + PAIRS]  # [64, 2]
    crd = np.stack([cs[0::2, 0], cs[0::2, 1], cs[1::2, 0], cs[1::2, 1]], 0)

    w1aT = np.ascontiguousarray(w1[:, :5].T)  # [5, 64]
    w1bT = np.ascontiguousarray(w1[:, 5:].T)  # [2, 64]
    wu = np.concatenate([w1aT, w1aT], 1)  # [5, 128]
    wc = np.zeros((4, 128), np.float32)
    wc[0:2, 0:64] = w1bT
    wc[2:4, 64:128] = w1bT

    wall = np.zeros((128, 4 * 128), np.float32)
    bball = np.zeros((128, 4), np.float32)
    for i, (w, bias) in enumerate(zip(ws, bs)):
        wall[0:64, 128 * i : 128 * i + 64] = w.T
        wall[64:128, 128 * i + 64 : 128 * i + 128] = w.T
        bball[:, i] = np.concatenate([bias, bias])

    wh = np.zeros((128, 6), np.float32)
    wh[0:64, 0:3] = wl.T / HW
    wh[64:128, 3:6] = wl.T / HW

    return {
        "xin": np.ascontiguousarray(xin).astype(np.float16),
        "crd": np.ascontiguousarray(crd).astype(np.float16),
        "wu": np.ascontiguousarray(wu).astype(np.float16),
        "wc": wc.astype(np.float16),
        "wall": wall.astype(np.float16),
        "bball": bball,
        "bb1": np.concatenate([b1, b1]).reshape(128, 1).astype(np.float32),
        "wh": wh,
        "bbl": np.concatenate([bl, bl]).reshape(6, 1).astype(np.float32),
    }


def _run(inputs, trace=False):
    image = np.asarray(inputs["image"], np.float32)
    coords = np.asarray(inputs["coords"], np.float32)
    w1 = np.asarray(inputs["w1"], np.float32)
    b1 = np.asarray(inputs["b1"], np.float32)
    ws = [np.asarray(inputs[f"w{i}"], np.float32) for i in (2, 3, 4, 5)]
    bs = [np.asarray(inputs[f"b{i}"], np.float32) for i in (2, 3, 4, 5)]
    wl = np.asarray(inputs["wl"], np.float32)
    bl = np.asarray(inputs["bl"], np.float32)

    nc = _get_nc()
    in_maps = [
        _prep_core_inputs(image, coords, w1, b1, ws, bs, wl, bl, c)
        for c in range(NCORES)
    ]
    res = run_bass_kernel_spmd(nc, in_maps, list(range(NCORES)), trace=trace)

    pred = np.empty((B, 3, N), np.float32)
    for c in range(NCORES):
        b = c // 2
        n0 = (c % 2) * PAIRS
        o = res.results[c]["out"]  # [6, 32]
        pred[b, :, n0 + 0 : n0 + PAIRS : 2] = o[0:3]
        pred[b, :, n0 + 1 : n0 + PAIRS : 2] = o[3:6]
    return pred, res


def kernel(**inputs) -> np.ndarray:
    pred, _ = _run(inputs, trace=False)
    return pred
